# revision 31
# baseline (speedup 1.0000x reference)
"""GQA attention block (B=2, S=2048, DIM=4096, 32 Q heads / 8 KV heads, HD=128,
RoPE + causal softmax + output projection) on 8 trn2 NeuronCores.

Sharding: 8 cores = 2 batches x 4 head-groups. Core c handles batch c%2 and
head-group c//2 (8 Q heads, 2 KV heads). Each core computes a full-size
[S, DIM] partial of the output projection (its heads' contribution); the host
sums the 4 group-partials per batch.

v2 vs the fp32r baseline:
  - all matmul operands in bf16 (same 1 cycle/row PE rate, half the DMA and
    SBUF footprint). PSUM accumulation stays fp32.
  - wq fits SBUF in bf16, so the Q projection accumulates the full 4096-dim
    contraction in one PSUM pass per token chunk: x streams from HBM once per
    phase instead of 4x, and the SBUF accumulation adds disappear.
  - softmax denominators: instead of a PE ones-matmul per key tile (512 rows
    each, ~8% of PE time), exp tiles are accumulated elementwise on the Vector
    engine and reduced with ONE ones-matmul per (qc, h).
  - causal diagonal tiles: scores/exp/PV are column-trimmed to the unmasked
    range (bf16 has no 256-wide full-rate requirement).
  - output projection (pure PE work) is interleaved into the attention loop
    one column-chunk per head, so the PE stays fed while the Scalar engine's
    exp (654ns/tile vs 426ns of PE work) lags.
  - W-phase PSUM->SBUF copies run on GpSimd (otherwise idle) so they don't
    contend with exp on Scalar.
"""

import math
import os
import sys
from contextlib import ExitStack
from dataclasses import dataclass

import numpy as np

sys.path.insert(0, "/opt/trn_rl_repo")

import concourse.bass as bass  # noqa: E402
import concourse.mybir as mybir  # noqa: E402
import concourse.tile as tile  # noqa: E402
from concourse import bacc  # noqa: E402

F32 = mybir.dt.float32
F32R = mybir.dt.float32r
BF16 = mybir.dt.bfloat16
P = 128


@dataclass(frozen=True)
class Cfg:
    S: int = 2048      # sequence length
    DIM: int = 4096    # model dim (contraction for projections)
    NH_L: int = 8      # q heads per core
    NKV_L: int = 2     # kv heads per core
    HD: int = 128      # head dim (must be P)
    TQ: int = 512      # token/query chunk (PSUM free dim)

    @property
    def CCH(self):  # contraction chunks
        return self.DIM // P

    @property
    def NT(self):  # token chunks
        return self.S // self.TQ

    @property
    def NKT(self):  # key tiles
        return self.S // P

    @property
    def RT(self):  # key tiles per token chunk
        return self.TQ // P

    @property
    def NREP(self):
        return self.NH_L // self.NKV_L


def build_program(cfg: Cfg, debug: bool = False) -> bass.Bass:
    nc = bacc.Bacc("TRN2", target_bir_lowering=False)
    S, DIM, NH_L, NKV_L, HD, TQ = cfg.S, cfg.DIM, cfg.NH_L, cfg.NKV_L, cfg.HD, cfg.TQ
    CCH, NT, RT = cfg.CCH, cfg.NT, cfg.RT
    MULT = mybir.AluOpType.mult
    ADD = mybir.AluOpType.add

    xT_d = nc.dram_tensor("xT", [DIM, S], BF16, kind="ExternalInput")
    wq_d = nc.dram_tensor("wq", [DIM, NH_L * HD], BF16, kind="ExternalInput")
    wk_d = nc.dram_tensor("wk", [DIM, NKV_L * HD], BF16, kind="ExternalInput")
    wv_d = nc.dram_tensor("wv", [DIM, NKV_L * HD], BF16, kind="ExternalInput")
    wo_d = nc.dram_tensor("wo", [NH_L * HD, DIM], BF16, kind="ExternalInput")
    cosq_d = nc.dram_tensor("cosq", [P, S], BF16, kind="ExternalInput")
    sinq_d = nc.dram_tensor("sinq", [P, S], BF16, kind="ExternalInput")
    cosk_d = nc.dram_tensor("cosk", [P, S], BF16, kind="ExternalInput")
    sink_d = nc.dram_tensor("sink", [P, S], BF16, kind="ExternalInput")
    triT_d = nc.dram_tensor("triT", [P, P], BF16, kind="ExternalInput")
    out_d = nc.dram_tensor("out", [S, DIM], BF16, kind="ExternalOutput")
    if debug:
        dbg_k = nc.dram_tensor("dbg_k", [P, NKV_L, S], BF16,
                               kind="ExternalOutput")
        dbg_v = nc.dram_tensor("dbg_v", [P, S // P, NKV_L * HD], BF16,
                               kind="ExternalOutput")
        dbg_q = nc.dram_tensor("dbg_q", [P, NH_L, S], BF16,
                               kind="ExternalOutput")
        dbg_a = nc.dram_tensor("dbg_a", [P, NH_L, S], BF16,
                               kind="ExternalOutput")

    xT_r = xT_d.ap().rearrange("(co ci) t -> ci co t", ci=P)
    wq_r = wq_d.ap().rearrange("(co ci) d -> ci co d", ci=P)
    wk_r = wk_d.ap().rearrange("(co ci) d -> ci co d", ci=P)
    wv_r = wv_d.ap().rearrange("(co ci) d -> ci co d", ci=P)
    wo_r = wo_d.ap().rearrange("(dc p) m -> p dc m", p=P)

    def mm(out, lhsT, rhs, start, stop):
        nc.tensor.matmul(out, lhsT, rhs, start=start, stop=stop)

    with tile.TileContext(nc) as tc, ExitStack() as top:
        # ---- whole-kernel residents -----------------------------------
        const = top.enter_context(tc.tile_pool(name="const", bufs=1))
        triT_sb = const.tile([P, P], BF16)
        ones_col = const.tile([P, 1], BF16)
        nc.vector.memset(ones_col[:], 1.0)
        ones_row = const.tile([1, P], BF16)
        nc.vector.memset(ones_row[:], 1.0)
        cosk_sb = const.tile([P, S], BF16)
        sink_sb = const.tile([P, S], BF16)
        cosq_sb = const.tile([P, S], BF16)
        sinq_sb = const.tile([P, S], BF16)
        KT_sb = const.tile([P, NKV_L, S], BF16)
        V_sb = const.tile([P, cfg.NKT, NKV_L * HD], BF16)
        qt_sb = const.tile([P, NH_L, S], BF16)

        def rope_inplace(dst, cos_sl, sin_sl, tmp_pool):
            # dst [P, n] bf16 in SBUF: dst = dst*cos + swap_halves(dst)*sin
            n = dst.shape[-1]
            tmp = tmp_pool.tile([P, TQ], BF16, tag="ropetmp", name="ropetmp")
            t = tmp[:, :n]
            nc.sync.dma_start(t[0:64], dst[64:128])
            nc.sync.dma_start(t[64:128], dst[0:64])
            nc.vector.tensor_tensor(t, t, sin_sl, MULT)
            nc.vector.tensor_tensor(dst, dst, cos_sl, MULT)
            nc.vector.tensor_tensor(dst, dst, t, ADD)

        # wq resident (right stack), prefetched during phase A
        wqp = tc.alloc_tile_pool(name="wqp", bufs=1, side="right")
        wq_sb = wqp.tile([P, CCH, NH_L * HD], BF16)

        # x tiles shared by phases A and Q (prefetch across the boundary)
        xstk = ExitStack()
        xap = xstk.enter_context(tc.tile_pool(name="xap", bufs=10))

        # ---------------- Phase A: K^T and V projections (+ RoPE on K) -----
        with ExitStack() as ctx:
            wkvp = ctx.enter_context(tc.tile_pool(name="wkvp", bufs=1))
            rtp = ctx.enter_context(tc.tile_pool(name="rtp", bufs=2))
            pka = ctx.enter_context(tc.tile_pool(name="pka", bufs=1, space="PSUM"))
            pva = ctx.enter_context(tc.tile_pool(name="pva", bufs=1, space="PSUM"))

            wk_sb = wkvp.tile([P, CCH, NKV_L * HD], BF16)
            wv_sb = wkvp.tile([P, CCH, NKV_L * HD], BF16)
            # first pieces small so the first matmul's weights arrive fast
            for i in (0, 2):
                nc.scalar.dma_start(wk_sb[:, i:i + 2, :], wk_r[:, i:i + 2, :])
                nc.scalar.dma_start(wv_sb[:, i:i + 2, :], wv_r[:, i:i + 2, :])
            for i in range(4, CCH, 4):
                nc.scalar.dma_start(wk_sb[:, i:i + 4, :], wk_r[:, i:i + 4, :])
                nc.scalar.dma_start(wv_sb[:, i:i + 4, :], wv_r[:, i:i + 4, :])
            # tables are first needed by the rope at the end of tn=0
            nc.scalar.dma_start(cosk_sb[:], cosk_d.ap())
            nc.scalar.dma_start(sink_sb[:], sink_d.ap())
            nc.scalar.dma_start(cosq_sb[:], cosq_d.ap())
            nc.scalar.dma_start(sinq_sb[:], sinq_d.ap())
            nc.scalar.dma_start(triT_sb[:], triT_d.ap())

            for tn in range(NT):
                tsl = slice(tn * TQ, (tn + 1) * TQ)
                psk = [pka.tile([P, TQ], F32, tag=f"psk{d}", name=f"psk{d}",
                                space="PSUM") for d in range(NKV_L)]
                # one full PSUM bank per token block: a start=True matmul
                # resets the whole bank, so regions can't share one.
                psv = [pva.tile([P, TQ], F32, tag=f"psv{j}", name=f"psv{j}",
                                space="PSUM") for j in range(RT)]
                for ci in range(CCH // 4):
                    if ci % 2 == 0:
                        # spread the wq prefetch across phase A: issuing it
                        # all up front oversubscribes HBM and starves the
                        # x-tile stream
                        i = tn * (CCH // 4) + ci
                        nc.scalar.dma_start(wq_sb[:, i:i + 2, :],
                                            wq_r[:, i:i + 2, :])
                    xa = xap.tile([P, 4, TQ], BF16, tag="xa", name="xa")
                    nc.sync.dma_start(xa[:], xT_r[:, ci * 4:(ci + 1) * 4, tsl])
                    for cc in range(4):
                        c = ci * 4 + cc
                        xt = xa[:, cc, :]
                        st, sp = c == 0, c == CCH - 1
                        for d in range(NKV_L):
                            mm(psk[d][:], wk_sb[:, c, d * HD:(d + 1) * HD],
                               xt, st, sp)
                        for j in range(RT):
                            mm(psv[j][:, 0:NKV_L * HD],
                               xt[:, j * P:(j + 1) * P], wv_sb[:, c, :], st, sp)
                for j in range(RT):
                    nc.scalar.copy(V_sb[:, tn * RT + j, :],
                                   psv[j][:, 0:NKV_L * HD])
                for d in range(NKV_L):
                    nc.scalar.copy(KT_sb[:, d, tsl], psk[d][:])
                    rope_inplace(KT_sb[:, d, tsl], cosk_sb[:, tsl],
                                 sink_sb[:, tsl], rtp)
                if tn == NT - 1:
                    # prefetch phase Q's first x tiles across the boundary
                    qpre = []
                    for ci in range(2):
                        t = xap.tile([P, 4, TQ], BF16, tag="xa", name="xa")
                        nc.sync.dma_start(t[:],
                                          xT_r[:, ci * 4:(ci + 1) * 4, 0:TQ])
                        qpre.append(t)

        # ---------------- Phase Q: Q^T projection (+ RoPE on Q) ------------
        with ExitStack() as ctx:
            rtq = ctx.enter_context(tc.tile_pool(name="rtq", bufs=2))
            pqa = ctx.enter_context(tc.tile_pool(name="pqa", bufs=1, space="PSUM"))

            for tn in range(NT):
                tsl = slice(tn * TQ, (tn + 1) * TQ)
                psq = [pqa.tile([P, TQ], F32, tag=f"psq{h}", name=f"psq{h}",
                                space="PSUM") for h in range(NH_L)]
                for ci in range(CCH // 4):
                    if tn == 0 and ci < 2:
                        xa = qpre[ci]
                    else:
                        xa = xap.tile([P, 4, TQ], BF16, tag="xa", name="xa")
                        nc.sync.dma_start(xa[:],
                                          xT_r[:, ci * 4:(ci + 1) * 4, tsl])
                    for cc in range(4):
                        c = ci * 4 + cc
                        st, sp = c == 0, c == CCH - 1
                        for h in range(NH_L):
                            mm(psq[h][:], wq_sb[:, c, h * HD:(h + 1) * HD],
                               xa[:, cc, :], st, sp)
                for h in range(NH_L):
                    if h % 2 == 0:
                        nc.vector.tensor_copy(qt_sb[:, h, tsl], psq[h][:])
                    else:
                        nc.scalar.copy(qt_sb[:, h, tsl], psq[h][:])
                    rope_inplace(qt_sb[:, h, tsl], cosq_sb[:, tsl],
                                 sinq_sb[:, tsl], rtq)
        xstk.close()
        wqp.release()

        if debug:
            nc.sync.dma_start(dbg_k.ap(), KT_sb[:])
            nc.sync.dma_start(dbg_v.ap(), V_sb[:])
            nc.sync.dma_start(dbg_q.ap(), qt_sb[:])

        # ---------------- Phase S+W: attention + output projection --------
        # wo resident (right stack), loaded at phase start, mc-major so the
        # first output-column chunks arrive first.
        wop = tc.alloc_tile_pool(name="wop", bufs=1, side="right")
        wo_sb = wop.tile([P, NH_L, DIM], BF16)
        atp = tc.alloc_tile_pool(name="atp", bufs=1, side="right")
        attnT_sb = atp.tile([P, NH_L, S], BF16)

        with ExitStack() as ctx:
            ptp = ctx.enter_context(tc.tile_pool(name="ptp", bufs=6))
            sap = ctx.enter_context(tc.tile_pool(name="sap", bufs=3))
            bcp = ctx.enter_context(tc.tile_pool(name="bcp", bufs=2))
            owp = ctx.enter_context(tc.tile_pool(name="owp", bufs=4))
            psc = ctx.enter_context(tc.tile_pool(name="psc", bufs=2, space="PSUM"))
            pso = ctx.enter_context(tc.tile_pool(name="pso", bufs=2, space="PSUM"))
            pdn = ctx.enter_context(tc.tile_pool(name="pdn", bufs=1, space="PSUM"))
            pbc = ctx.enter_context(tc.tile_pool(name="pbc", bufs=1, space="PSUM"))
            psw = ctx.enter_context(tc.tile_pool(name="psw", bufs=2, space="PSUM"))

            for m0 in range(0, DIM, TQ):
                for i in range(0, NH_L, 2):
                    nc.sync.dma_start(wo_sb[:, i:i + 2, m0:m0 + TQ],
                                      wo_r[:, i:i + 2, m0:m0 + TQ])

            def s_main(qc, h, filler=None):
                """kt loop for one (qc, h): returns (acc_v, ps_out).

                filler: list of thunks each issuing a short burst of PE work
                (output-projection chunks); popped every few kt iterations to
                keep the PE fed while the Scalar engine's exp lags."""
                g = h // cfg.NREP
                q0 = qc * TQ
                acc_v = sap.tile([P, TQ], BF16, tag="sacc", name="sacc")
                ps_out = pso.tile([P, TQ], F32, tag="psout", name="psout",
                                  space="PSUM")
                # diagonal tiles first (qoff=0 initializes full width), then
                # off-diagonal; the last instruction is always full width so
                # PSUM start/stop flags cover every element.
                kts = list(range(qc * RT, (qc + 1) * RT)) + list(range(0, qc * RT))
                n = len(kts)
                for i, kt in enumerate(kts):
                    diag = kt >= qc * RT
                    qoff = (kt - qc * RT) * P if diag else 0
                    cols = slice(qoff, TQ)
                    ps_sc = psc.tile([P, TQ], F32, tag="pssc", name="pssc",
                                     space="PSUM")
                    mm(ps_sc[:, cols], KT_sb[:, g, kt * P:(kt + 1) * P],
                       qt_sb[:, h, q0 + qoff:q0 + TQ], True, True)
                    pt = ptp.tile([P, TQ], BF16, tag="pt", name="pt")
                    nc.scalar.activation(pt[:, cols], ps_sc[:, cols],
                                         mybir.ActivationFunctionType.Exp)
                    if diag:
                        # zero the masked upper triangle of the diagonal block
                        # post-exp (bf16 2x DVE; off the exp critical path)
                        nc.vector.tensor_tensor(pt[:, qoff:qoff + P],
                                                pt[:, qoff:qoff + P],
                                                triT_sb[:], MULT)
                    first, last = i == 0, i == n - 1
                    pv_cols = cols
                    if last and qoff > 0:
                        # stop-flag instruction must be full width; zero the
                        # masked columns of pt so they contribute nothing.
                        nc.vector.memset(pt[:, 0:qoff], 0.0)
                        pv_cols = slice(0, TQ)
                    mm(ps_out[:, pv_cols], V_sb[:, kt, g * HD:(g + 1) * HD],
                       pt[:, pv_cols], first, last)
                    if first:
                        nc.vector.tensor_copy(acc_v[:], pt[:])
                    else:
                        nc.vector.tensor_tensor(acc_v[:, cols], acc_v[:, cols],
                                                pt[:, cols], ADD)
                    if filler and i % 4 == 3:
                        filler.pop(0)()
                while filler:
                    filler.pop(0)()
                return acc_v, ps_out

            def s_finish(qc, h, acc_v, ps_out):
                q0 = qc * TQ
                ps_den = pdn.tile([1, TQ], F32, tag="psden", name="psden",
                                  space="PSUM")
                mm(ps_den[:], ones_col[:], acc_v[:], True, True)
                rrow = bcp.tile([1, TQ], F32, tag="rrow", name="rrow")
                nc.vector.reciprocal_approx_fast(out=rrow[:], in_=ps_den[:])
                rrow_b = bcp.tile([1, TQ], BF16, tag="rrowb", name="rrowb")
                nc.vector.tensor_copy(rrow_b[:], rrow[:])
                # broadcast 1/den across partitions with a rank-1 PE matmul
                # (gpsimd partition_broadcast costs ~2.5us of software queue)
                ps_bc = pbc.tile([P, TQ], F32, tag="psbc", name="psbc",
                                 space="PSUM")
                mm(ps_bc[:], ones_row[:], rrow_b[:], True, True)
                bc_sb = bcp.tile([P, TQ], F32, tag="bcsb", name="bcsb")
                nc.vector.tensor_copy(bc_sb[:], ps_bc[:])
                nc.vector.tensor_tensor(attnT_sb[:, h, q0:q0 + TQ], ps_out[:],
                                        bc_sb[:], MULT)

            def w_tb(mc, tb):
                """one token-block of the output projection for column chunk
                mc: 8 PE matmuls + a PSUM->SBUF copy + the output DMA."""
                msl = slice(mc * TQ, (mc + 1) * TQ)
                ps_w = psw.tile([P, TQ], F32, tag="psw", name="psw",
                                space="PSUM")
                for dc in range(NH_L):
                    mm(ps_w[:], attnT_sb[:, dc, tb * P:(tb + 1) * P],
                       wo_sb[:, dc, msl], dc == 0, dc == NH_L - 1)
                ot = owp.tile([P, TQ], BF16, tag="ot", name="ot")
                if tb % 2 == 0:
                    nc.scalar.copy(ot[:], ps_w[:])
                else:
                    nc.vector.tensor_copy(ot[:], ps_w[:])
                nc.sync.dma_start(out_d.ap()[tb * P:(tb + 1) * P, msl], ot[:])

            def w_chunk(qc, mc):
                for tb in range(qc * RT, (qc + 1) * RT):
                    w_tb(mc, tb)

            for qc in range(NT):
                for h in range(NH_L):
                    if qc > 0:
                        mc = h
                        filler = [(lambda mc=mc, tb=tb: w_tb(mc, tb))
                                  for tb in range((qc - 1) * RT, qc * RT)]
                    else:
                        filler = []
                    acc, po = s_main(qc, h, filler)
                    s_finish(qc, h, acc, po)
            for mc in range(NH_L):
                w_chunk(NT - 1, mc)

            if debug:
                nc.sync.dma_start(dbg_a.ap(), attnT_sb[:])

        atp.release()
        wop.release()

    nc.compile()
    return nc


# ---------------------------------------------------------------------------
# Host side
# ---------------------------------------------------------------------------

_HALF_PERM = np.concatenate([np.arange(0, P, 2), np.arange(1, P, 2)])

LAST_EXEC_NS = None
LAST_RESULTS = None


def _host_prep(cfg: Cfg, x, wq, wk, wv, wo, freqs_cos, freqs_sin):
    """Build the 8 per-core input maps. Core c: batch c % 2, group c // 2."""
    import ml_dtypes
    bf16 = ml_dtypes.bfloat16

    B = x.shape[0]
    n_groups = wq.shape[1] // (cfg.NH_L * cfg.HD)
    hd = cfg.HD

    cosT = np.ascontiguousarray(freqs_cos.T.astype(np.float32))  # [HD/2, S]
    sinT = np.ascontiguousarray(freqs_sin.T.astype(np.float32))
    sc = np.float32(1.0 / math.sqrt(hd))
    cosq = (np.concatenate([cosT, cosT], 0) * sc).astype(bf16)
    sinq = (np.concatenate([-sinT, sinT], 0) * sc).astype(bf16)
    cosk = np.concatenate([cosT, cosT], 0).astype(bf16)
    sink = np.concatenate([-sinT, sinT], 0).astype(bf16)
    # triT[k, q] = 1 where key k is visible to query q within the diagonal
    # block (k <= q), 0 above; multiplied into the exp tile post-softmax-exp.
    triT = np.triu(np.ones((P, P), np.float32)).astype(bf16)

    xT = [np.ascontiguousarray(x[b].T).astype(bf16) for b in range(B)]

    def permute_cols(w, nheads):
        w = w.reshape(cfg.DIM, nheads, hd)[:, :, _HALF_PERM]
        return np.ascontiguousarray(w.reshape(cfg.DIM, nheads * hd)).astype(bf16)

    in_maps = []
    qcols = cfg.NH_L * hd
    kcols = cfg.NKV_L * hd
    for c in range(B * n_groups):
        b, g = c % B, c // B
        in_maps.append(dict(
            xT=xT[b],
            wq=permute_cols(wq[:, g * qcols:(g + 1) * qcols], cfg.NH_L),
            wk=permute_cols(wk[:, g * kcols:(g + 1) * kcols], cfg.NKV_L),
            wv=np.ascontiguousarray(wv[:, g * kcols:(g + 1) * kcols]).astype(bf16),
            wo=np.ascontiguousarray(wo[g * qcols:(g + 1) * qcols, :]).astype(bf16),
            cosq=cosq, sinq=sinq, cosk=cosk, sink=sink, triT=triT,
        ))
    return in_maps


def kernel(x, wq, wk, wv, wo, freqs_cos, freqs_sin, mask, start_pos=0):
    global LAST_EXEC_NS, LAST_RESULTS
    x = np.asarray(x, np.float32)
    wq = np.asarray(wq, np.float32)
    wk = np.asarray(wk, np.float32)
    wv = np.asarray(wv, np.float32)
    wo = np.asarray(wo, np.float32)
    freqs_cos = np.asarray(freqs_cos, np.float32)
    freqs_sin = np.asarray(freqs_sin, np.float32)

    cfg = Cfg()
    B = x.shape[0]
    n_groups = 4
    in_maps = _host_prep(cfg, x, wq, wk, wv, wo, freqs_cos, freqs_sin)

    from concourse.bass_utils import run_bass_kernel_spmd

    nc = build_program(cfg)
    trace = bool(int(os.environ.get("KERNEL_TRACE", "0")))
    res = run_bass_kernel_spmd(nc, in_maps, core_ids=list(range(len(in_maps))),
                               trace=trace)
    LAST_EXEC_NS = res.exec_time_ns
    LAST_RESULTS = res

    out = np.zeros((B, cfg.S, cfg.DIM), np.float32)
    for c in range(B * n_groups):
        b = c % B
        out[b] += res.results[c]["out"].astype(np.float32)
    return out
